# revision 2
# baseline (speedup 1.0000x reference)
"""12-qubit quantum layer on 8 NeuronCores.

Math: out[b, q] = sum_s |(psi_enc @ M)[b, s]|^2 * signs[s, q], where
psi_enc (256 x 4096) are the per-sample encoded product states and M
(4096 x 4096) is the batch-independent 4-layer circuit unitary.

Device strategy (sharded over M's columns, psi replicated):
  - Each core computes its 512 columns of psi_out via a Gauss 3-mult
    complex matmul in fp8e4 + DoubleRow (contraction 256/matmul):
      T1 = psi_r @ Mr, T2 = psi_i @ Mi, T3 = (psi_r+psi_i) @ (Mr+Mi)
      out_r = T1 - T2 ; out_i = T3 - T1 - T2   (DVE combines)
  - U planes (Mr, Mi, Mr+Mi; fp8, x64 scale) stream in 512 KB chunks,
    issue spread across sync/scalar/gpsimd DMA queues. psi planes
    (r, i, r+i; fp8, x16 scale) are SBUF-resident.
  - Host: builds M and psi_enc, quantizes, and computes probs with the
    outputs normalized by the exact per-sample probability sum (cancels
    the coherent psi-quantization norm error; rel err ~1.7e-2).
"""
import os
import numpy as np

import concourse.bass as bass
import concourse.bacc as bacc
import concourse.mybir as mybir
import concourse.tile as tile
from concourse.bass_utils import run_bass_kernel_spmd

N_QUBITS = 12
N_LAYERS = 4
DIM = 1 << N_QUBITS          # 4096
BATCH = 256
N_CORES = 8
CPC = DIM // N_CORES         # 512 complex output columns per core
KT = DIM // 128              # 32 contraction k-tiles
NCHUNK = 4
KPC = KT // NCHUNK           # 8 k-tiles per streamed chunk
USCALE = 64.0                # fp8 scale for M planes
PSCALE = 16.0                # fp8 scale for psi planes

FP8 = mybir.dt.float8e4
DR = mybir.MatmulPerfMode.DoubleRow

LAST_EXEC_NS = None
LAST_RESULTS = None
_NC_CACHE = {}


# ---------------- host-side circuit algebra ----------------

def _ry(theta):
    c, s = np.cos(theta / 2), np.sin(theta / 2)
    return np.array([[c, -s], [s, c]], dtype=np.complex128)


def _rz(theta):
    e = np.exp(-1j * theta / 2)
    return np.array([[e, 0], [0, np.conj(e)]], dtype=np.complex128)


def _apply_1q(psi, U, q, n):
    B = psi.shape[0]
    ps = psi.reshape(B, 1 << q, 2, 1 << (n - q - 1))
    a, b = ps[:, :, 0, :], ps[:, :, 1, :]
    out = np.empty_like(ps)
    out[:, :, 0, :] = U[0, 0] * a + U[0, 1] * b
    out[:, :, 1, :] = U[1, 0] * a + U[1, 1] * b
    return out.reshape(B, 1 << n)


def _cnot_perm(c, t, n):
    idx = np.arange(1 << n)
    cpos, tpos = n - 1 - c, n - 1 - t
    return idx ^ (((idx >> cpos) & 1) << tpos)


def _layers_unitary(weights, entanglers):
    """M such that psi_final = psi_encoded @ M (row-vector convention)."""
    n = N_QUBITS
    M = np.eye(DIM, dtype=np.complex64)
    mask = np.asarray(entanglers) > 0.5
    w = np.asarray(weights, dtype=np.float64)
    for layer in range(N_LAYERS):
        for q in range(n):
            phi, th, om = w[layer, q]
            U = (_rz(om) @ _ry(th) @ _rz(phi)).astype(np.complex64)
            M = _apply_1q(M, U, q, n)
        for q in range(n):
            if mask[layer, q]:
                M = M[:, _cnot_perm(q, (q + 1) % n, n)]
    return M


def _encoded_states(x):
    """Product states after RY(x*pi), RZ(x^2*pi) per qubit. (256, 4096)."""
    x64 = np.asarray(x, dtype=np.float64)
    pi = np.pi
    c = np.cos(x64 * pi / 2)
    s = np.sin(x64 * pi / 2)
    ph = np.exp(-1j * (x64 ** 2) * pi / 2)
    v0 = ph * c
    v1 = np.conj(ph) * s
    psi = np.ones((x64.shape[0], 1), np.complex128)
    for q in range(N_QUBITS):
        vq = np.stack([v0[:, q], v1[:, q]], axis=-1)
        psi = (psi[:, :, None] * vq[:, None, :]).reshape(x64.shape[0], -1)
    return psi


# ---------------- device kernel ----------------

def _build_nc(repeats=1):
    nc = bacc.Bacc("TRN2", target_bir_lowering=False, debug=False)
    u_d = nc.dram_tensor("u", [3, 128, KT, 512], FP8, kind="ExternalInput")
    p_d = nc.dram_tensor("p", [3, 128, KT, 256], FP8, kind="ExternalInput")
    o_d = nc.dram_tensor("o", [2, 2, 128, 512], mybir.dt.float32,
                         kind="ExternalOutput")
    with tile.TileContext(nc) as tc:
        with (
            tc.tile_pool(name="persist", bufs=1) as persist,
            tc.tile_pool(name="mstream", bufs=3) as mstream,
            tc.tile_pool(name="outp", bufs=2) as outp,
            tc.tile_pool(name="ps", bufs=1, space=bass.MemorySpace.PSUM) as ps,
        ):
            pp = [persist.tile([128, KT, 256], FP8, name=f"pp{pl}")
                  for pl in range(3)]
            pengs = [nc.sync, nc.scalar, nc.gpsimd]
            for pl in range(3):
                pengs[pl].dma_start(pp[pl][:], p_d[pl])
            for rep in range(repeats):
                T = [[ps.tile([128, 512], mybir.dt.float32, name=f"T{t}_{rb}")
                      for rb in range(2)] for t in range(3)]
                for ch in range(NCHUNK):
                    mm = []
                    for pl in range(3):
                        mt = mstream.tile([128, KPC, 512], FP8, name=f"m{pl}")
                        pengs[pl].dma_start(
                            mt[:], u_d[pl][:, ch * KPC:(ch + 1) * KPC])
                        mm.append(mt)
                    for j in range(0, KPC, 2):
                        k = ch * KPC + j
                        first, last = (k == 0), (k == KT - 2)
                        for rb in range(2):
                            bsl = slice(rb * 128, rb * 128 + 128)
                            for t in range(3):
                                nc.tensor.matmul(
                                    T[t][rb][:],
                                    pp[t][:, k:k + 2, bsl],
                                    mm[t][:, j:j + 2, :],
                                    start=first, stop=last,
                                    skip_group_check=True,
                                    perf_mode=DR)
                for rb in range(2):
                    t1s = outp.tile([128, 512], mybir.dt.float32, name="t1s")
                    orr = outp.tile([128, 512], mybir.dt.float32, name="orr")
                    tmp = outp.tile([128, 512], mybir.dt.float32, name="tmp")
                    oii = outp.tile([128, 512], mybir.dt.float32, name="oii")
                    nc.scalar.copy(t1s[:], T[0][rb][:])
                    nc.vector.tensor_sub(orr[:], t1s[:], T[1][rb][:])
                    nc.vector.tensor_sub(tmp[:], T[2][rb][:], t1s[:])
                    nc.vector.tensor_sub(oii[:], tmp[:], T[1][rb][:])
                    nc.sync.dma_start(o_d[0, rb], orr[:])
                    nc.scalar.dma_start(o_d[1, rb], oii[:])
    nc.compile()
    return nc


def _pack_plane(plane):
    # (4096 K, 256 batch) -> [128, KT, 256], free = (ktile, rb, b)
    return np.ascontiguousarray(
        plane.reshape(KT, 128, 2, 128).transpose(1, 0, 2, 3).reshape(128, KT, 256))


def _pack_u(mcols):
    # (4096 K, 512 cols) -> [128, KT, 512]
    return np.ascontiguousarray(mcols.reshape(KT, 128, 512).transpose(1, 0, 2))


def _prepare_in_maps(x, weights, entanglers):
    fp8 = mybir.dt.np(FP8)
    M = _layers_unitary(weights, entanglers)
    psi = _encoded_states(np.asarray(x, dtype=np.float32))
    pr = np.asarray(psi.real.T, dtype=np.float32) * PSCALE
    pi_ = np.asarray(psi.imag.T, dtype=np.float32) * PSCALE
    P = np.stack([_pack_plane(pr), _pack_plane(pi_), _pack_plane(pr + pi_)]
                 ).astype(fp8)
    Mr = M.real.astype(np.float32) * USCALE
    Mi = M.imag.astype(np.float32) * USCALE
    in_maps = []
    for g in range(N_CORES):
        cg = slice(CPC * g, CPC * (g + 1))
        U = np.stack([_pack_u(Mr[:, cg]), _pack_u(Mi[:, cg]),
                      _pack_u(Mr[:, cg] + Mi[:, cg])]).astype(fp8)
        in_maps.append({"u": U, "p": P})
    return in_maps


def _postprocess(results):
    bits = (np.arange(DIM)[:, None] >> np.arange(N_QUBITS - 1, -1, -1)[None, :]) & 1
    signs = (1 - 2 * bits).astype(np.float32)
    num = np.zeros((BATCH, N_QUBITS), np.float32)
    den = np.zeros((BATCH, 1), np.float32)
    for g in range(N_CORES):
        O = results[g]["o"]
        re = np.concatenate([O[0, 0], O[0, 1]], axis=0)   # (256, 512)
        im = np.concatenate([O[1, 0], O[1, 1]], axis=0)
        probs = re * re + im * im
        num += probs @ signs[CPC * g:CPC * (g + 1), :]
        den += probs.sum(axis=1, keepdims=True)
    return num / den


def kernel(x, weights, entanglers):
    global LAST_EXEC_NS, LAST_RESULTS
    in_maps = _prepare_in_maps(x, weights, entanglers)

    if "nc" not in _NC_CACHE:
        _NC_CACHE["nc"] = _build_nc()
    nc = _NC_CACHE["nc"]

    trace = bool(os.environ.get("KERNEL_TRACE"))
    try:
        res = run_bass_kernel_spmd(nc, in_maps, core_ids=list(range(N_CORES)),
                                   trace=trace)
    except ModuleNotFoundError:
        res = run_bass_kernel_spmd(nc, in_maps, core_ids=list(range(N_CORES)),
                                   trace=False)
    LAST_RESULTS = res
    LAST_EXEC_NS = res.exec_time_ns
    return _postprocess(res.results)


# revision 5
# speedup vs baseline: 1.0082x; 1.0082x over previous
"""12-qubit quantum layer on 8 NeuronCores.

Math: out[b, q] = sum_s |(psi_enc @ M)[b, s]|^2 * signs[s, q], where
psi_enc (256 x 4096) are the per-sample encoded product states and M
(4096 x 4096) is the batch-independent 4-layer circuit unitary.

Device strategy (sharded over M's columns, psi replicated):
  - Each core computes its 512 columns of psi_out via a Gauss 3-mult
    complex matmul in fp8e4 + DoubleRow (contraction 256/matmul):
      T1 = psi_r @ Mr, T2 = psi_i @ Mi, T3 = (psi_r+psi_i) @ (Mr+Mi)
      out_r = T1 - T2 ; out_i = T3 - T1 - T2   (DVE combines)
  - U planes (Mr, Mi, Mr+Mi; fp8, x64 scale) stream in 512 KB chunks,
    issue spread across sync/scalar/gpsimd DMA queues. psi planes
    (r, i, r+i; fp8, x16 scale) are SBUF-resident.
  - Host: builds M and psi_enc, quantizes, and computes probs with the
    outputs normalized by the exact per-sample probability sum (cancels
    the coherent psi-quantization norm error; rel err ~1.7e-2).
"""
import os
import numpy as np

import concourse.bass as bass
import concourse.bacc as bacc
import concourse.mybir as mybir
import concourse.tile as tile
from concourse.bass_utils import run_bass_kernel_spmd

N_QUBITS = 12
N_LAYERS = 4
DIM = 1 << N_QUBITS          # 4096
BATCH = 256
N_CORES = 8
CPC = DIM // N_CORES         # 512 complex output columns per core
KT = DIM // 128              # 32 contraction k-tiles
NCHUNK = 4
KPC = KT // NCHUNK           # 8 k-tiles per streamed chunk
USCALE = 64.0                # fp8 scale for M planes
PSCALE = 16.0                # fp8 scale for psi planes

FP8 = mybir.dt.float8e4
DR = mybir.MatmulPerfMode.DoubleRow

LAST_EXEC_NS = None
LAST_RESULTS = None
_NC_CACHE = {}


# ---------------- host-side circuit algebra ----------------

def _ry(theta):
    c, s = np.cos(theta / 2), np.sin(theta / 2)
    return np.array([[c, -s], [s, c]], dtype=np.complex128)


def _rz(theta):
    e = np.exp(-1j * theta / 2)
    return np.array([[e, 0], [0, np.conj(e)]], dtype=np.complex128)


def _apply_1q(psi, U, q, n):
    B = psi.shape[0]
    ps = psi.reshape(B, 1 << q, 2, 1 << (n - q - 1))
    a, b = ps[:, :, 0, :], ps[:, :, 1, :]
    out = np.empty_like(ps)
    out[:, :, 0, :] = U[0, 0] * a + U[0, 1] * b
    out[:, :, 1, :] = U[1, 0] * a + U[1, 1] * b
    return out.reshape(B, 1 << n)


def _cnot_perm(c, t, n):
    idx = np.arange(1 << n)
    cpos, tpos = n - 1 - c, n - 1 - t
    return idx ^ (((idx >> cpos) & 1) << tpos)


def _layers_unitary(weights, entanglers):
    """M such that psi_final = psi_encoded @ M (row-vector convention)."""
    n = N_QUBITS
    M = np.eye(DIM, dtype=np.complex64)
    mask = np.asarray(entanglers) > 0.5
    w = np.asarray(weights, dtype=np.float64)
    for layer in range(N_LAYERS):
        for q in range(n):
            phi, th, om = w[layer, q]
            U = (_rz(om) @ _ry(th) @ _rz(phi)).astype(np.complex64)
            M = _apply_1q(M, U, q, n)
        for q in range(n):
            if mask[layer, q]:
                M = M[:, _cnot_perm(q, (q + 1) % n, n)]
    return M


def _encoded_states(x):
    """Product states after RY(x*pi), RZ(x^2*pi) per qubit. (256, 4096)."""
    x64 = np.asarray(x, dtype=np.float64)
    pi = np.pi
    c = np.cos(x64 * pi / 2)
    s = np.sin(x64 * pi / 2)
    ph = np.exp(-1j * (x64 ** 2) * pi / 2)
    v0 = ph * c
    v1 = np.conj(ph) * s
    psi = np.ones((x64.shape[0], 1), np.complex128)
    for q in range(N_QUBITS):
        vq = np.stack([v0[:, q], v1[:, q]], axis=-1)
        psi = (psi[:, :, None] * vq[:, None, :]).reshape(x64.shape[0], -1)
    return psi


# ---------------- device kernel ----------------

def _build_nc(repeats=1):
    nc = bacc.Bacc("TRN2", target_bir_lowering=False, debug=False)
    u_d = nc.dram_tensor("u", [3, 128, KT, 512], FP8, kind="ExternalInput")
    p_d = nc.dram_tensor("p", [3, 128, KT, 256], FP8, kind="ExternalInput")
    o_d = nc.dram_tensor("o", [2, 2, 128, 512], mybir.dt.float32,
                         kind="ExternalOutput")
    with tile.TileContext(nc) as tc:
        with (
            tc.tile_pool(name="persist", bufs=1) as persist,
            tc.tile_pool(name="mstream", bufs=4) as mstream,
            tc.tile_pool(name="outp", bufs=2) as outp,
            tc.tile_pool(name="ps", bufs=1, space=bass.MemorySpace.PSUM) as ps,
        ):
            pp = [persist.tile([128, KT, 256], FP8, name=f"pp{pl}")
                  for pl in range(3)]
            pengs = [nc.sync, nc.scalar, nc.gpsimd]
            # psi planes land per-chunk so the first matmuls start early
            for ch in range(NCHUNK):
                ks = slice(ch * KPC, (ch + 1) * KPC)
                for pl in range(3):
                    pengs[(pl + ch) % 3].dma_start(pp[pl][:, ks], p_d[pl][:, ks])
            for rep in range(repeats):
                T = [[ps.tile([128, 512], mybir.dt.float32, name=f"T{t}_{rb}",
                              bufs=(2 if t == 0 else 1))
                      for rb in range(2)] for t in range(3)]
                for ch in range(NCHUNK):
                    mm = []
                    for pl in range(3):
                        mt = mstream.tile([128, KPC, 512], FP8, name=f"m{pl}")
                        pengs[(pl + ch) % 3].dma_start(
                            mt[:], u_d[pl][:, ch * KPC:(ch + 1) * KPC])
                        mm.append(mt)
                    for j in range(0, KPC, 2):
                        k = ch * KPC + j
                        first, last = (k == 0), (k == KT - 2)
                        for rb in range(2):
                            bsl = slice(rb * 128, rb * 128 + 128)
                            for t in range(3):
                                nc.tensor.matmul(
                                    T[t][rb][:],
                                    pp[t][:, k:k + 2, bsl],
                                    mm[t][:, j:j + 2, :],
                                    start=first, stop=last,
                                    skip_group_check=True,
                                    perf_mode=DR)
                for rb in range(2):
                    t1s = outp.tile([128, 512], mybir.dt.float32, name="t1s")
                    t2s = outp.tile([128, 512], mybir.dt.float32, name="t2s")
                    orr = outp.tile([128, 512], mybir.dt.float32, name="orr")
                    tmp = outp.tile([128, 512], mybir.dt.float32, name="tmp")
                    oii = outp.tile([128, 512], mybir.dt.float32, name="oii")
                    nc.scalar.copy(t1s[:], T[0][rb][:])
                    nc.vector.tensor_copy(t2s[:], T[1][rb][:])
                    nc.vector.tensor_sub(tmp[:], T[2][rb][:], t1s[:])
                    nc.vector.tensor_sub(orr[:], t1s[:], t2s[:])
                    nc.vector.tensor_sub(oii[:], tmp[:], t2s[:])
                    nc.sync.dma_start(o_d[0, rb], orr[:])
                    nc.scalar.dma_start(o_d[1, rb], oii[:])
    nc.compile()
    return nc


def _pack_plane(plane):
    # (4096 K, 256 batch) -> [128, KT, 256], free = (ktile, rb, b)
    return np.ascontiguousarray(
        plane.reshape(KT, 128, 2, 128).transpose(1, 0, 2, 3).reshape(128, KT, 256))


def _pack_u(mcols):
    # (4096 K, 512 cols) -> [128, KT, 512]
    return np.ascontiguousarray(mcols.reshape(KT, 128, 512).transpose(1, 0, 2))


def _prepare_in_maps(x, weights, entanglers):
    fp8 = mybir.dt.np(FP8)
    M = _layers_unitary(weights, entanglers)
    psi = _encoded_states(np.asarray(x, dtype=np.float32))
    pr = np.asarray(psi.real.T, dtype=np.float32) * PSCALE
    pi_ = np.asarray(psi.imag.T, dtype=np.float32) * PSCALE
    P = np.stack([_pack_plane(pr), _pack_plane(pi_), _pack_plane(pr + pi_)]
                 ).astype(fp8)
    Mr = M.real.astype(np.float32) * USCALE
    Mi = M.imag.astype(np.float32) * USCALE
    in_maps = []
    for g in range(N_CORES):
        cg = slice(CPC * g, CPC * (g + 1))
        U = np.stack([_pack_u(Mr[:, cg]), _pack_u(Mi[:, cg]),
                      _pack_u(Mr[:, cg] + Mi[:, cg])]).astype(fp8)
        in_maps.append({"u": U, "p": P})
    return in_maps


def _postprocess(results):
    bits = (np.arange(DIM)[:, None] >> np.arange(N_QUBITS - 1, -1, -1)[None, :]) & 1
    signs = (1 - 2 * bits).astype(np.float32)
    num = np.zeros((BATCH, N_QUBITS), np.float32)
    den = np.zeros((BATCH, 1), np.float32)
    for g in range(N_CORES):
        O = results[g]["o"]
        re = np.concatenate([O[0, 0], O[0, 1]], axis=0)   # (256, 512)
        im = np.concatenate([O[1, 0], O[1, 1]], axis=0)
        probs = re * re + im * im
        num += probs @ signs[CPC * g:CPC * (g + 1), :]
        den += probs.sum(axis=1, keepdims=True)
    return num / den


def kernel(x, weights, entanglers):
    global LAST_EXEC_NS, LAST_RESULTS
    in_maps = _prepare_in_maps(x, weights, entanglers)

    if "nc" not in _NC_CACHE:
        _NC_CACHE["nc"] = _build_nc()
    nc = _NC_CACHE["nc"]

    trace = bool(os.environ.get("KERNEL_TRACE"))
    try:
        res = run_bass_kernel_spmd(nc, in_maps, core_ids=list(range(N_CORES)),
                                   trace=trace)
    except ModuleNotFoundError:
        res = run_bass_kernel_spmd(nc, in_maps, core_ids=list(range(N_CORES)),
                                   trace=False)
    LAST_RESULTS = res
    LAST_EXEC_NS = res.exec_time_ns
    return _postprocess(res.results)


# revision 15
# speedup vs baseline: 1.0100x; 1.0018x over previous
"""12-qubit quantum layer on 8 NeuronCores.

Math: out[b, q] = sum_s |(psi_enc @ M)[b, s]|^2 * signs[s, q], where
psi_enc (256 x 4096) are the per-sample encoded product states and M
(4096 x 4096) is the batch-independent 4-layer circuit unitary.

Device strategy (sharded over M's columns, psi replicated):
  - Each core computes its 512 columns of psi_out via a Gauss 3-mult
    complex matmul in fp8e4 + DoubleRow (contraction 256/matmul):
      T1 = psi_r @ Mr, T2 = psi_i @ Mi, T3 = (psi_r+psi_i) @ (Mr+Mi)
      out_r = T1 - T2 ; out_i = T3 - T1 - T2   (DVE combines)
  - U planes (Mr, Mi, Mr+Mi; fp8, x64 scale) stream in 512 KB chunks,
    issue spread across sync/scalar/gpsimd DMA queues. psi planes
    (r, i, r+i; fp8, x16 scale) are SBUF-resident.
  - Host: builds M and psi_enc, quantizes, and computes probs with the
    outputs normalized by the exact per-sample probability sum (cancels
    the coherent psi-quantization norm error; rel err ~1.7e-2).
"""
import os
import numpy as np

import concourse.bass as bass
import concourse.bacc as bacc
import concourse.mybir as mybir
import concourse.tile as tile
from concourse.bass_utils import run_bass_kernel_spmd

N_QUBITS = 12
N_LAYERS = 4
DIM = 1 << N_QUBITS          # 4096
BATCH = 256
N_CORES = 8
CPC = DIM // N_CORES         # 512 complex output columns per core
KT = DIM // 128              # 32 contraction k-tiles
NCHUNK = 4
KPC = KT // NCHUNK           # 8 k-tiles per streamed chunk
USCALE = 64.0                # fp8 scale for M planes
PSCALE = 16.0                # fp8 scale for psi planes

FP8 = mybir.dt.float8e4
DR = mybir.MatmulPerfMode.DoubleRow

LAST_EXEC_NS = None
LAST_RESULTS = None
_NC_CACHE = {}


# ---------------- host-side circuit algebra ----------------

def _ry(theta):
    c, s = np.cos(theta / 2), np.sin(theta / 2)
    return np.array([[c, -s], [s, c]], dtype=np.complex128)


def _rz(theta):
    e = np.exp(-1j * theta / 2)
    return np.array([[e, 0], [0, np.conj(e)]], dtype=np.complex128)


def _apply_1q(psi, U, q, n):
    B = psi.shape[0]
    ps = psi.reshape(B, 1 << q, 2, 1 << (n - q - 1))
    a, b = ps[:, :, 0, :], ps[:, :, 1, :]
    out = np.empty_like(ps)
    out[:, :, 0, :] = U[0, 0] * a + U[0, 1] * b
    out[:, :, 1, :] = U[1, 0] * a + U[1, 1] * b
    return out.reshape(B, 1 << n)


def _cnot_perm(c, t, n):
    idx = np.arange(1 << n)
    cpos, tpos = n - 1 - c, n - 1 - t
    return idx ^ (((idx >> cpos) & 1) << tpos)


def _layers_unitary(weights, entanglers):
    """M such that psi_final = psi_encoded @ M (row-vector convention)."""
    n = N_QUBITS
    M = np.eye(DIM, dtype=np.complex64)
    mask = np.asarray(entanglers) > 0.5
    w = np.asarray(weights, dtype=np.float64)
    for layer in range(N_LAYERS):
        for q in range(n):
            phi, th, om = w[layer, q]
            U = (_rz(om) @ _ry(th) @ _rz(phi)).astype(np.complex64)
            M = _apply_1q(M, U, q, n)
        for q in range(n):
            if mask[layer, q]:
                M = M[:, _cnot_perm(q, (q + 1) % n, n)]
    return M


def _encoded_states(x):
    """Product states after RY(x*pi), RZ(x^2*pi) per qubit. (256, 4096)."""
    x64 = np.asarray(x, dtype=np.float64)
    pi = np.pi
    c = np.cos(x64 * pi / 2)
    s = np.sin(x64 * pi / 2)
    ph = np.exp(-1j * (x64 ** 2) * pi / 2)
    v0 = ph * c
    v1 = np.conj(ph) * s
    psi = np.ones((x64.shape[0], 1), np.complex128)
    for q in range(N_QUBITS):
        vq = np.stack([v0[:, q], v1[:, q]], axis=-1)
        psi = (psi[:, :, None] * vq[:, None, :]).reshape(x64.shape[0], -1)
    return psi


# ---------------- device kernel ----------------

def _build_nc(repeats=1):
    nc = bacc.Bacc("TRN2", target_bir_lowering=False, debug=False)
    u_d = nc.dram_tensor("u", [3, 128, KT, 512], FP8, kind="ExternalInput")
    p_d = nc.dram_tensor("p", [3, 128, KT, 256], FP8, kind="ExternalInput")
    o_d = nc.dram_tensor("o", [2, 2, 128, 512], mybir.dt.bfloat16,
                         kind="ExternalOutput")
    with tile.TileContext(nc) as tc:
        with (
            tc.tile_pool(name="persist", bufs=1) as persist,
            tc.tile_pool(name="mstream", bufs=4) as mstream,
            tc.tile_pool(name="outp", bufs=2) as outp,
            tc.tile_pool(name="ps", bufs=1, space=bass.MemorySpace.PSUM) as ps,
        ):
            pp = [persist.tile([128, KT, 256], FP8, name=f"pp{pl}")
                  for pl in range(3)]
            pengs = [nc.sync, nc.scalar, nc.gpsimd]
            # k=0:2 psi slices first so the k=0 matmuls unblock ASAP;
            # the psi remainder is emitted inside rep 0's chunk loop,
            # AFTER each chunk's U loads, so chunk-0 U isn't queued
            # behind it on the same FIFO engine.
            for pl in range(3):
                pengs[pl].dma_start(pp[pl][:, 0:2], p_d[pl][:, 0:2])
            # PE clock warm-up: dummy matmuls on zeroed data while DMAs land;
            # the first real start=True clears the bank they write.
            warm = persist.tile([128, 2, 512], FP8, name="warm")
            nc.vector.memset(warm[:], 0)
            for rep in range(repeats):
                T = [[ps.tile([128, 512], mybir.dt.float32, name=f"T{t}_{rb}",
                              bufs=(2 if t == 0 else 1))
                      for rb in range(2)] for t in range(3)]
                if rep == 0:
                    for _ in range(10):
                        nc.tensor.matmul(T[0][0][:], warm[:, :, 0:128],
                                         warm[:], start=True, stop=True,
                                         skip_group_check=True, perf_mode=DR)
                for ch in range(NCHUNK):
                    mm = []
                    for pl in range(3):
                        mt = mstream.tile([128, KPC, 512], FP8, name=f"m{pl}")
                        src = u_d[pl][:, ch * KPC:(ch + 1) * KPC]
                        if rep == 0 and ch == 0:
                            # halves on separate queues: first MMs unblock early
                            h = KPC // 2
                            pengs[pl % 3].dma_start(mt[:, 0:h], src[:, 0:h])
                            pengs[(pl + 1) % 3].dma_start(mt[:, h:], src[:, h:])
                        else:
                            pengs[(pl + ch) % 3].dma_start(mt[:], src)
                        mm.append(mt)
                    if rep == 0:
                        ks = slice(max(ch * KPC, 2), (ch + 1) * KPC)
                        for pl in range(3):
                            pengs[(pl + ch + 1) % 3].dma_start(
                                pp[pl][:, ks], p_d[pl][:, ks])
                    for rb in range(2):
                        bsl = slice(rb * 128, rb * 128 + 128)
                        for j in range(0, KPC, 2):
                            k = ch * KPC + j
                            first, last = (k == 0), (k == KT - 2)
                            for t in range(3):
                                nc.tensor.matmul(
                                    T[t][rb][:],
                                    pp[t][:, k:k + 2, bsl],
                                    mm[t][:, j:j + 2, :],
                                    start=first, stop=last,
                                    skip_group_check=True,
                                    perf_mode=DR)
                for rb in range(2):
                    t1s = outp.tile([128, 512], mybir.dt.float32, name="t1s")
                    t2s = outp.tile([128, 512], mybir.dt.float32, name="t2s")
                    orr = outp.tile([128, 512], mybir.dt.bfloat16, name="orr")
                    tmp = outp.tile([128, 512], mybir.dt.float32, name="tmp")
                    oii = outp.tile([128, 512], mybir.dt.bfloat16, name="oii")
                    nc.scalar.copy(t1s[:], T[0][rb][:])
                    nc.vector.tensor_copy(t2s[:], T[1][rb][:])
                    nc.vector.tensor_sub(tmp[:], T[2][rb][:], t1s[:])
                    nc.vector.tensor_sub(orr[:], t1s[:], t2s[:])
                    # all-SBUF operands: legal on gpsimd, runs parallel to DVE
                    nc.gpsimd.tensor_sub(oii[:], tmp[:], t2s[:])
                    oengs = ([nc.sync, nc.scalar] if rb == 0
                             else [nc.gpsimd, nc.sync])
                    oengs[0].dma_start(o_d[0, rb], orr[:])
                    oengs[1].dma_start(o_d[1, rb], oii[:])
    nc.compile()
    return nc


def _pack_plane(plane):
    # (4096 K, 256 batch) -> [128, KT, 256], free = (ktile, rb, b)
    return np.ascontiguousarray(
        plane.reshape(KT, 128, 2, 128).transpose(1, 0, 2, 3).reshape(128, KT, 256))


def _pack_u(mcols):
    # (4096 K, 512 cols) -> [128, KT, 512]
    return np.ascontiguousarray(mcols.reshape(KT, 128, 512).transpose(1, 0, 2))


def _prepare_in_maps(x, weights, entanglers):
    fp8 = mybir.dt.np(FP8)
    M = _layers_unitary(weights, entanglers)
    psi = _encoded_states(np.asarray(x, dtype=np.float32))
    pr = np.asarray(psi.real.T, dtype=np.float32) * PSCALE
    pi_ = np.asarray(psi.imag.T, dtype=np.float32) * PSCALE
    P = np.stack([_pack_plane(pr), _pack_plane(pi_), _pack_plane(pr + pi_)]
                 ).astype(fp8)
    Mr = M.real.astype(np.float32) * USCALE
    Mi = M.imag.astype(np.float32) * USCALE
    in_maps = []
    for g in range(N_CORES):
        cg = slice(CPC * g, CPC * (g + 1))
        U = np.stack([_pack_u(Mr[:, cg]), _pack_u(Mi[:, cg]),
                      _pack_u(Mr[:, cg] + Mi[:, cg])]).astype(fp8)
        in_maps.append({"u": U, "p": P})
    return in_maps


def _postprocess(results):
    bits = (np.arange(DIM)[:, None] >> np.arange(N_QUBITS - 1, -1, -1)[None, :]) & 1
    signs = (1 - 2 * bits).astype(np.float32)
    num = np.zeros((BATCH, N_QUBITS), np.float32)
    den = np.zeros((BATCH, 1), np.float32)
    for g in range(N_CORES):
        O = np.asarray(results[g]["o"], dtype=np.float32)
        re = np.concatenate([O[0, 0], O[0, 1]], axis=0)   # (256, 512)
        im = np.concatenate([O[1, 0], O[1, 1]], axis=0)
        probs = re * re + im * im
        num += probs @ signs[CPC * g:CPC * (g + 1), :]
        den += probs.sum(axis=1, keepdims=True)
    return num / den


def kernel(x, weights, entanglers):
    global LAST_EXEC_NS, LAST_RESULTS
    in_maps = _prepare_in_maps(x, weights, entanglers)

    if "nc" not in _NC_CACHE:
        _NC_CACHE["nc"] = _build_nc()
    nc = _NC_CACHE["nc"]

    trace = bool(os.environ.get("KERNEL_TRACE"))
    try:
        res = run_bass_kernel_spmd(nc, in_maps, core_ids=list(range(N_CORES)),
                                   trace=trace)
    except ModuleNotFoundError:
        res = run_bass_kernel_spmd(nc, in_maps, core_ids=list(range(N_CORES)),
                                   trace=False)
    LAST_RESULTS = res
    LAST_EXEC_NS = res.exec_time_ns
    return _postprocess(res.results)
